# revision 1
# baseline (speedup 1.0000x reference)
"""AveragePrevEmbeddingsLM Trainium2 kernel (8 NeuronCores, vocab-sharded).

logits[b, t, v] = mean(emb_table[x[b, :t+1]]) @ W.T + b_vec

Strategy: shard the vocab dim across 8 cores (4000 each). Every core
redundantly gathers + prefix-sums all 8192 token embeddings (cheap),
then computes its (8192 x 64) @ (64 x 4000) logits slice. The 1 GB
logits write is the memory roofline (~131 MB/core).

Device pipeline per core:
  dma_gather (emb rows, per batch)  -> [128tok, 16blk, 64emb] SBUF
  PE transpose per 128-token block  -> [64emb, 128tok] PSUM -> SBUF seg
  tensor_tensor_scan along seq      -> causal prefix sums Y
  per 128-token tile: matmul(lhsT=[Y; pos+1], rhs=[W.T; bias]) -> PSUM
  ScalarE scaled copy (x 1/(pos+1)) -> SBUF -> 2MB DMA out

The bias is folded in via an extra contraction row (lhsT row 64 =
pos+1, rhs row 64 = bias); dividing by pos+1 on the PSUM->SBUF copy
then yields mean-pooled logits + bias exactly.
"""

import os
import sys

import numpy as np

for _p in ("/opt/trn_rl_repo",):
    if _p not in sys.path and os.path.isdir(_p):
        sys.path.append(_p)

VOCAB, EMB, B, SEQ = 32000, 64, 4, 2048
NCORES = 8
VS = VOCAB // NCORES       # vocab shard per core
TOK = B * SEQ
BLK = SEQ // 128           # 128-token blocks per batch row
MTILES = TOK // 128
NCHUNK = 8
CHUNK = VS // NCHUNK       # matmul free-dim chunk (one PSUM bank)

COMPUTE = os.environ.get("KERNEL_COMPUTE", "f32r")   # f32r | f32 | bf16
K_ROWS = int(os.environ.get("KERNEL_K_ROWS", "65"))  # 65 (exact) or 128 (padded)

_prog_cache = {}


def _build(compute: str, k_rows: int):
    from concourse import bacc
    import concourse.mybir as mybir
    import concourse.tile as tile
    from concourse.masks import make_identity

    f32 = mybir.dt.float32
    cdt = {
        "f32r": mybir.dt.float32r,
        "f32": f32,
        "bf16": mybir.dt.bfloat16,
    }[compute]

    nc = bacc.Bacc(None, target_bir_lowering=False)

    emb_d = nc.dram_tensor("emb", [VOCAB, EMB], f32, kind="ExternalInput")
    idx_d = nc.dram_tensor("idx", [128, TOK // 128], mybir.dt.int32, kind="ExternalInput")
    wtb_d = nc.dram_tensor("wtb", [128, VS], f32, kind="ExternalInput")
    posp1_d = nc.dram_tensor("posp1", [1, SEQ], f32, kind="ExternalInput")
    recip_d = nc.dram_tensor("recip", [128, BLK], f32, kind="ExternalInput")
    out_d = nc.dram_tensor("out", [TOK, VS], f32, kind="ExternalOutput")

    with tile.TileContext(nc) as tc:
        with (
            tc.tile_pool(name="const", bufs=1) as constp,
            tc.tile_pool(name="gath", bufs=2) as gathp,
            tc.tile_pool(name="segraw", bufs=2) as segrawp,
            tc.tile_pool(name="segcum", bufs=2) as segcump,
            tc.tile_pool(name="outp", bufs=6) as outp,
            tc.tile_pool(name="ptr", bufs=1, space="PSUM") as ptrp,
            tc.tile_pool(name="pmm", bufs=7, space="PSUM") as pmmp,
        ):
            wtb_sb = constp.tile([128, VS], f32)
            nc.sync.dma_start(wtb_sb[:], wtb_d[:])
            recip_sb = constp.tile([128, BLK], f32)
            nc.sync.dma_start(recip_sb[:], recip_d[:])
            idx_sb = constp.tile([128, TOK // 128], mybir.dt.int32)
            nc.sync.dma_start(idx_sb[:], idx_d[:])
            ident = constp.tile([128, 128], f32)
            make_identity(nc, ident[:])

            if cdt == f32:
                wtb_c = wtb_sb[:]
            else:
                wtb_cast = constp.tile([128, VS], cdt)
                nc.vector.tensor_copy(wtb_cast[:], wtb_sb[:])
                wtb_c = wtb_cast[:]

            import concourse.bass as bass

            # Software pipeline at 512-token (4 m-tile) "quarter"
            # granularity: head(Q) = gather + PE-transpose + chained scan
            # (+ cast); proj(Q) = 4 m-tiles of matmul + scaled copy + DMA
            # out. head(Q+1) is emitted before proj(Q) so each engine's
            # in-order stream interleaves next-quarter prep with current
            # projections.
            QT = 4                      # m-tiles per quarter
            NQ = MTILES // QT           # total quarters (16)
            QSEQ = QT * 128             # tokens per quarter (512)
            state = {}

            def head(Q):
                b, q = Q // (BLK // QT), Q % (BLK // QT)
                if q == 0:
                    state["gath"] = gathp.tile([128, BLK, EMB], f32, tag="gath", name="gath")
                    state["seg_raw"] = segrawp.tile([EMB, SEQ], f32, tag="seg_raw", name="seg_raw")
                    state["seg_cum"] = segcump.tile([k_rows, SEQ], f32, tag="seg_cum", name="seg_cum")
                    nc.sync.dma_start(
                        state["seg_cum"][EMB:EMB + 1, :], posp1_d[:])
                    if k_rows > EMB + 1:
                        nc.vector.memset(
                            state["seg_cum"][EMB + 1:k_rows, :], 0.0)
                    if cdt != f32:
                        state["seg_cast"] = segcump.tile(
                            [k_rows, SEQ], cdt, tag="segcast", name="segcast")
                        if k_rows > EMB:
                            nc.vector.tensor_copy(
                                state["seg_cast"][EMB:k_rows, :],
                                state["seg_cum"][EMB:k_rows, :])
                gath, seg_raw = state["gath"], state["seg_raw"]
                seg_cum = state["seg_cum"]
                for mb in range(q * QT, (q + 1) * QT):
                    m = b * BLK + mb
                    nc.gpsimd.indirect_dma_start(
                        out=gath[:, mb, :],
                        out_offset=None,
                        in_=emb_d[:],
                        in_offset=bass.IndirectOffsetOnAxis(
                            ap=idx_sb[:, m:m + 1], axis=0,
                        ),
                    )
                    pt = ptrp.tile([EMB, 128], f32)
                    nc.tensor.transpose(pt[:], gath[:, mb, :], ident[:])
                    nc.vector.tensor_copy(
                        seg_raw[:, mb * 128:(mb + 1) * 128], pt[:])
                qsl = slice(q * QSEQ, (q + 1) * QSEQ)
                initial = (0.0 if q == 0 else
                           seg_cum[0:EMB, q * QSEQ - 1:q * QSEQ])
                nc.vector.tensor_tensor_scan(
                    seg_cum[0:EMB, qsl],
                    seg_raw[0:EMB, qsl],
                    seg_raw[0:EMB, qsl],
                    initial,
                    op0=mybir.AluOpType.add,
                    op1=mybir.AluOpType.bypass,
                )
                if cdt != f32:
                    nc.vector.tensor_copy(
                        state["seg_cast"][0:EMB, qsl], seg_cum[0:EMB, qsl])
                    state["seg_c"] = state["seg_cast"][:]
                else:
                    state["seg_c"] = seg_cum[:]

            def proj(Q, seg_c):
                b, q = Q // (BLK // QT), Q % (BLK // QT)
                for mb in range(q * QT, (q + 1) * QT):
                    m = b * BLK + mb
                    otile = outp.tile([128, NCHUNK, CHUNK], f32)
                    lhsT = seg_c[:, mb * 128:(mb + 1) * 128]
                    scale = recip_sb[:, mb:mb + 1]
                    # 8 single-bank PSUM tiles (bank = 512 f32), one
                    # N=500 matmul each, then per-chunk scaled copy,
                    # alternating ACT/DVE.
                    for ch in range(NCHUNK):
                        ps = pmmp.tile([128, 512], f32)
                        nc.tensor.matmul(
                            ps[:, 0:CHUNK],
                            lhsT,
                            wtb_c[0:k_rows, ch * CHUNK:(ch + 1) * CHUNK],
                            start=True,
                            stop=True,
                        )
                        osl = otile[:, ch, :]
                        if ch % 8 != 1 and ch % 8 != 4 and ch % 8 != 6:
                            nc.scalar.activation(
                                osl, ps[:, 0:CHUNK],
                                mybir.ActivationFunctionType.Copy,
                                scale=scale,
                            )
                        else:
                            nc.vector.tensor_scalar_mul(
                                osl, ps[:, 0:CHUNK], scale)
                        if ch == 3:
                            nc.sync.dma_start(
                                out_d[m * 128:(m + 1) * 128, 0:VS // 2],
                                otile[:, 0:NCHUNK // 2, :])
                        elif ch == NCHUNK - 1:
                            nc.sync.dma_start(
                                out_d[m * 128:(m + 1) * 128, VS // 2:VS],
                                otile[:, NCHUNK // 2:NCHUNK, :])


            LEAD = 1
            seg_of = {}
            for Q in range(min(LEAD, NQ)):
                head(Q)
                seg_of[Q] = state["seg_c"]
            for Q in range(NQ):
                if Q + LEAD < NQ:
                    head(Q + LEAD)
                    seg_of[Q + LEAD] = state["seg_c"]
                proj(Q, seg_of.pop(Q))

    nc.compile()
    return nc


def _get_prog(compute: str, k_rows: int):
    key = (compute, k_rows)
    if key not in _prog_cache:
        _prog_cache[key] = _build(compute, k_rows)
    return _prog_cache[key]


def _make_in_maps(emb_table, W, b, x):
    emb_table = np.ascontiguousarray(np.asarray(emb_table, dtype=np.float32))
    W = np.asarray(W, dtype=np.float32)
    b = np.asarray(b, dtype=np.float32)
    x = np.asarray(x).astype(np.int64).reshape(B, SEQ)

    # idx layout: token m*128 + p -> idx[p, m]
    wrapped = np.ascontiguousarray(
        x.reshape(-1).reshape(TOK // 128, 128).T.astype(np.int32)
    )

    posp1 = np.arange(1, SEQ + 1, dtype=np.float32)[None, :]
    i = np.arange(128)[:, None]
    mb = np.arange(BLK)[None, :]
    recip = (1.0 / (mb * 128 + i + 1)).astype(np.float32)

    in_maps = []
    for c in range(NCORES):
        wtb = np.zeros((128, VS), dtype=np.float32)
        wtb[0:EMB] = W[c * VS:(c + 1) * VS, :].T
        wtb[EMB] = b[c * VS:(c + 1) * VS]
        in_maps.append({
            "emb": emb_table,
            "idx": wrapped,
            "wtb": np.ascontiguousarray(wtb),
            "posp1": posp1,
            "recip": recip,
        })
    return in_maps


def kernel(emb_table, W, b, x, trace=False):
    from concourse.bass_utils import run_bass_kernel_spmd

    nc = _get_prog(COMPUTE, K_ROWS)
    in_maps = _make_in_maps(emb_table, W, b, x)
    res = run_bass_kernel_spmd(
        nc, in_maps, core_ids=list(range(NCORES)), trace=trace,
    )

    out = np.empty((TOK, VOCAB), dtype=np.float32)
    for c in range(NCORES):
        out[:, c * VS:(c + 1) * VS] = res.results[c]["out"]
    out = out.reshape(B, SEQ, VOCAB)
    if trace:
        return out, res
    return out



# revision 9
# speedup vs baseline: 1.1707x; 1.1707x over previous
"""AveragePrevEmbeddingsLM Trainium2 kernel (8 NeuronCores, vocab-sharded).

logits[b, t, v] = mean(emb_table[x[b, :t+1]]) @ W.T + b_vec

Strategy: shard the vocab dim across 8 cores (4000 each). Every core
redundantly gathers all 8192 token embeddings and computes causal
prefix sums (cheap), then computes its (8192 x 64) @ (64 x 4000)
logits slice. The logits DMA-out is the memory roofline, so logits
are written as bf16 (quantization rel-err ~1e-3, far under the 2e-2
gate) and upcast to f32 on the host: ~65.5 MB/core instead of 131 MB.

Device pipeline per core:
  dma_gather (emb rows, per 128-token block) -> [128tok, 16blk, 64] SBUF
  prefix matmul: out[e,t] = sum_k gath[k,e] * U[k,t], U upper-tri ones
    -> fuses transpose + block-local prefix sum in one PE op (PSUM)
  DVE carry add (+ prefix total of previous block) -> seg_f32 SBUF
  per 512-token quarter: cast seg_f32 -> seg bf16 (lhsT for matmuls)
  per 128-token m-tile: 8 matmuls (bf16, N=500) -> PSUM banks
  ACT/DVE multi-bank scaled copies (x 1/(pos+1)) -> bf16 SBUF -> DMA out

The bias is folded in via an extra contraction row (lhsT row 64 =
pos+1 in bf16, rhs row 64 = bias); dividing by pos+1 on the
PSUM->SBUF copy then yields mean-pooled logits + bias.
"""

import os
import sys

import numpy as np

for _p in ("/opt/trn_rl_repo",):
    if _p not in sys.path and os.path.isdir(_p):
        sys.path.append(_p)

VOCAB, EMB, B, SEQ = 32000, 64, 4, 2048
NCORES = 8
VS = VOCAB // NCORES       # vocab shard per core
TOK = B * SEQ
BLK = SEQ // 128           # 128-token blocks per batch row
MTILES = TOK // 128
NCHUNK = 8
CHUNK = VS // NCHUNK       # matmul free-dim chunk (one PSUM bank)
K_ROWS = EMB + 1           # 64 emb rows + 1 bias row

_prog_cache = {}


def _build():
    from concourse import bacc
    import concourse.mybir as mybir
    import concourse.tile as tile

    f32 = mybir.dt.float32
    bf16 = mybir.dt.bfloat16

    nc = bacc.Bacc(None, target_bir_lowering=False)

    emb_d = nc.dram_tensor("emb", [VOCAB, EMB], f32, kind="ExternalInput")
    idx_d = nc.dram_tensor("idx", [128, MTILES], mybir.dt.int32, kind="ExternalInput")
    wtb_d = nc.dram_tensor("wtb", [K_ROWS, VS], bf16, kind="ExternalInput")
    posp1_d = nc.dram_tensor("posp1", [1, TOK], bf16, kind="ExternalInput")
    umat_d = nc.dram_tensor("umat", [128, 128], f32, kind="ExternalInput")
    recip_d = nc.dram_tensor("recip", [128, BLK], f32, kind="ExternalInput")
    out_d = nc.dram_tensor("out", [TOK, VS], bf16, kind="ExternalOutput")

    with tile.TileContext(nc) as tc:
        with (
            tc.tile_pool(name="const", bufs=1) as constp,
            tc.tile_pool(name="gath", bufs=2) as gathp,
            tc.tile_pool(name="outp", bufs=6) as outp,
            tc.tile_pool(name="pfx", bufs=2, space="PSUM") as pfxp,
            tc.tile_pool(name="pmm", bufs=2, space="PSUM") as pmmp,
        ):
            wtb_sb = constp.tile([K_ROWS, VS], bf16)
            nc.sync.dma_start(wtb_sb[:], wtb_d[:])
            recip_sb = constp.tile([128, BLK], f32)
            nc.sync.dma_start(recip_sb[:], recip_d[:])
            idx_sb = constp.tile([128, MTILES], mybir.dt.int32)
            nc.sync.dma_start(idx_sb[:], idx_d[:])
            umat = constp.tile([128, 128], f32)
            nc.sync.dma_start(umat[:], umat_d[:])

            # seg: prefix sums. f32 master copy for exact carries, bf16
            # cast used as matmul lhsT. Row 64 of the bf16 seg is the
            # bias row (pos+1).
            seg_f = constp.tile([EMB, SEQ], f32)
            seg_b = constp.tile([K_ROWS, TOK], bf16)
            nc.sync.dma_start(seg_b[EMB:EMB + 1, :], posp1_d[:])

            # prefix PSUM: per-block [64, 128] tiles, each padded to a
            # full bank; bufs=2 alternates banks so the carry-add read
            # of block i never shares a bank with the PE write of
            # block i+1 (same-bank PE-W + DVE-R is fatal on TRN2).

            import concourse.bass as bass

            # Software pipeline at 512-token (4 m-tile) "quarter"
            # granularity: head(Q) = gather + prefix-matmul + carry adds
            # + bf16 cast; proj(Q) = 4 m-tiles of matmuls + scaled
            # multi-bank copies + DMA out. head(Q+1) is emitted before
            # proj(Q).
            QT = 4                      # m-tiles per quarter
            NQ = MTILES // QT           # total quarters (16)
            QSEQ = QT * 128             # tokens per quarter (512)
            state = {}

            def head(Q):
                b, q = Q // (BLK // QT), Q % (BLK // QT)
                if q == 0:
                    state["gath"] = gathp.tile(
                        [128, BLK, EMB], f32, tag="gath", name="gath")
                gath = state["gath"]
                for i in range(QT):
                    mb = q * QT + i
                    m = b * BLK + mb
                    nc.gpsimd.indirect_dma_start(
                        out=gath[:, mb, :],
                        out_offset=None,
                        in_=emb_d[:],
                        in_offset=bass.IndirectOffsetOnAxis(
                            ap=idx_sb[:, m:m + 1], axis=0,
                        ),
                    )
                    # block-local prefix sums, transposed:
                    # pq[e, t] = sum_{k<=t} gath[k, e]
                    pq = pfxp.tile([EMB, 128], f32, tag="pfx", name="pq")
                    nc.tensor.matmul(
                        pq[:], gath[:, mb, :], umat[:],
                        start=True, stop=True,
                    )
                    scol = slice(mb * 128, (mb + 1) * 128)
                    if mb == 0:
                        nc.vector.tensor_copy(seg_f[:, scol], pq[:])
                    else:
                        nc.vector.tensor_scalar_add(
                            seg_f[:, scol], pq[:],
                            seg_f[:, mb * 128 - 1: mb * 128],
                        )
                qsl = slice(b * SEQ + q * QSEQ, b * SEQ + (q + 1) * QSEQ)
                fsl = slice(q * QSEQ, (q + 1) * QSEQ)
                nc.vector.tensor_copy(seg_b[0:EMB, qsl], seg_f[:, fsl])

            def proj(Q):
                b, q = Q // (BLK // QT), Q % (BLK // QT)
                for i in range(QT):
                    mb = q * QT + i
                    m = b * BLK + mb
                    otile = outp.tile([128, NCHUNK, CHUNK], bf16)
                    lhsT = seg_b[:, b * SEQ + mb * 128: b * SEQ + (mb + 1) * 128]
                    scale = recip_sb[:, mb:mb + 1]
                    # chunk groups: A = ch 0-2 (ACT), B = ch 3-5 (DVE),
                    # C = ch 6-7 (ACT). Each group is one multi-bank
                    # PSUM tile drained by a single copy instruction.
                    ga = pmmp.tile([128, 3, 512], f32, tag="pmm", name="ga")
                    for j in range(3):
                        ch = j
                        nc.tensor.matmul(
                            ga[:, j, 0:CHUNK], lhsT,
                            wtb_sb[:, ch * CHUNK:(ch + 1) * CHUNK],
                            start=True, stop=True,
                        )
                    nc.scalar.activation(
                        otile[:, 0:3, :], ga[:, :, 0:CHUNK],
                        mybir.ActivationFunctionType.Copy,
                        scale=scale,
                    )
                    gb = pmmp.tile([128, 3, 512], f32, tag="pmm", name="gb")
                    for j in range(3):
                        ch = 3 + j
                        nc.tensor.matmul(
                            gb[:, j, 0:CHUNK], lhsT,
                            wtb_sb[:, ch * CHUNK:(ch + 1) * CHUNK],
                            start=True, stop=True,
                        )
                    nc.vector.tensor_scalar_mul(
                        otile[:, 3:6, :], gb[:, :, 0:CHUNK], scale)
                    gc = pmmp.tile([128, 3, 512], f32, tag="pmm", name="gc")
                    for j in range(2):
                        ch = 6 + j
                        nc.tensor.matmul(
                            gc[:, j, 0:CHUNK], lhsT,
                            wtb_sb[:, ch * CHUNK:(ch + 1) * CHUNK],
                            start=True, stop=True,
                        )
                    nc.scalar.activation(
                        otile[:, 6:8, :], gc[:, 0:2, 0:CHUNK],
                        mybir.ActivationFunctionType.Copy,
                        scale=scale,
                    )
                    nc.sync.dma_start(
                        out_d[m * 128:(m + 1) * 128, 0:3 * CHUNK],
                        otile[:, 0:3, :])
                    nc.sync.dma_start(
                        out_d[m * 128:(m + 1) * 128, 3 * CHUNK:VS],
                        otile[:, 3:8, :])

            LEAD = 1
            for Q in range(min(LEAD, NQ)):
                head(Q)
            for Q in range(NQ):
                if Q + LEAD < NQ:
                    head(Q + LEAD)
                proj(Q)

    nc.compile()
    return nc


def _get_prog():
    if "v2" not in _prog_cache:
        _prog_cache["v2"] = _build()
    return _prog_cache["v2"]


def _make_in_maps(emb_table, W, b, x):
    import ml_dtypes

    bf = ml_dtypes.bfloat16
    emb_table = np.ascontiguousarray(np.asarray(emb_table, dtype=np.float32))
    W = np.asarray(W, dtype=np.float32)
    b = np.asarray(b, dtype=np.float32)
    x = np.asarray(x).astype(np.int64).reshape(B, SEQ)

    # idx layout: token m*128 + p -> idx[p, m]
    wrapped = np.ascontiguousarray(
        x.reshape(-1).reshape(MTILES, 128).T.astype(np.int32)
    )

    posp1 = np.tile(
        np.arange(1, SEQ + 1, dtype=np.float32), B)[None, :].astype(bf)
    umat = np.ascontiguousarray(np.triu(np.ones((128, 128), np.float32)))
    i = np.arange(128)[:, None]
    mb = np.arange(BLK)[None, :]
    recip = (1.0 / (mb * 128 + i + 1)).astype(np.float32)

    in_maps = []
    for c in range(NCORES):
        wtb = np.zeros((K_ROWS, VS), dtype=np.float32)
        wtb[0:EMB] = W[c * VS:(c + 1) * VS, :].T
        wtb[EMB] = b[c * VS:(c + 1) * VS]
        in_maps.append({
            "emb": emb_table,
            "idx": wrapped,
            "wtb": np.ascontiguousarray(wtb.astype(bf)),
            "posp1": posp1,
            "umat": umat,
            "recip": recip,
        })
    return in_maps


def kernel(emb_table, W, b, x, trace=False):
    from concourse.bass_utils import run_bass_kernel_spmd

    nc = _get_prog()
    in_maps = _make_in_maps(emb_table, W, b, x)
    res = run_bass_kernel_spmd(
        nc, in_maps, core_ids=list(range(NCORES)), trace=trace,
    )

    out = np.empty((TOK, VOCAB), dtype=np.float32)
    for c in range(NCORES):
        out[:, c * VS:(c + 1) * VS] = np.asarray(
            res.results[c]["out"]).astype(np.float32)
    out = out.reshape(B, SEQ, VOCAB)
    if trace:
        return out, res
    return out
